# revision 19
# baseline (speedup 1.0000x reference)
"""BallQLoss kernel for 8 Trainium2 NeuronCores.

Computes mean_{b,i,k} |flow[b,i] - flow[b, idx[b,i,k]]|_1 where idx are the
first K=16 in-ball (radius 0.5) neighbors of each point in index order,
padded with the first neighbor (pointnet2 ball_query semantics).

Sharding: data-parallel over (B x N): each of 8 cores takes 2048 queries of
one batch element and holds the full 8192-point replica of that batch.

Queries are sorted by |q|^2 on the host (densest first) and dealt so that
row-tile t on every core holds queries of the same density band. Selection
then only scans a per-band prefix P_t of the index axis; a free tail-check
(ACT relu-sum accumulator) verifies on device that no query needed points
beyond its prefix, and the host falls back to an exact numpy computation in
that (never observed) case.

Per row-tile pipeline:
  PE    : score = (r^2 - d^2)/2 via an augmented 5-dim matmul (fp32)
  ACT   : relu(score * 1e30) -> f16 {inf, 0} + per-block accum for the tail
  DVE   : keys = min(relu16, iota_desc) (f16 2x); per-2048-chunk max8 ->
          match_replace -> max8 = first-16; rebase, merge, idx = N - key
  GPSIMD: per-slot indirect-DMA gather of neighbor flows
  DVE   : L1 diff reduce; partition-reduce partials at the end
"""

import numpy as np
from contextlib import ExitStack

K = 16
RADIUS = 0.5
B = 2
N = 8192
N_CORES = 8
QPC = (B * N) // N_CORES  # 2048 queries per core
RT = 128                  # queries per row-tile (SBUF partition dim)
NRT = QPC // RT           # 16 row-tiles per core
CHUNK = 2048              # fp16-exact local iota range
BLK = 512                 # PSUM bank width (fp32)

# Prefix length per density band (band = row-tile index after sorting by
# |q|^2 ascending within each batch). Measured max "needed prefix" on the
# reference input distribution + >=256 margin, rounded up to 512. The device
# verifies sufficiency at runtime; host falls back to numpy if flagged.
P_BANDS = [1536, 1536, 2048, 2048, 2048, 2560, 3072, 3584,
           3584, 4608, 5120, 7168, 8192, 8192, 8192, 8192]

_cached = None


def _build_program(repeat=1, mm_bf16=False, do_gather=True):
    import concourse.bass as bass
    import concourse.tile as tile
    from concourse import bacc, bass_isa, mybir

    f32 = mybir.dt.float32
    f16 = mybir.dt.float16
    i32 = mybir.dt.int32
    u16 = mybir.dt.uint16
    Alu = mybir.AluOpType
    Act = mybir.ActivationFunctionType

    nc = bacc.Bacc("TRN2", target_bir_lowering=False, debug=False,
                   num_devices=N_CORES)

    at = nc.dram_tensor("at", [5, QPC], f32, kind="ExternalInput").ap()
    bt = nc.dram_tensor("bt", [5, N], f32, kind="ExternalInput").ap()
    flowall = nc.dram_tensor("flowall", [N, 3], f32, kind="ExternalInput").ap()
    flowq = nc.dram_tensor("flowq", [QPC, 3], f32, kind="ExternalInput").ap()
    partial = nc.dram_tensor("partial", [1, 1], f32, kind="ExternalOutput").ap()
    flags = nc.dram_tensor("flags", [1, 1], f32, kind="ExternalOutput").ap()

    with tile.TileContext(nc) as tc, ExitStack() as ctx:
        cpool = ctx.enter_context(tc.tile_pool(name="const", bufs=1))
        kpool = ctx.enter_context(tc.tile_pool(name="keys", bufs=2))
        ppool = ctx.enter_context(tc.tile_pool(name="ps", bufs=4, space="PSUM"))
        spool = ctx.enter_context(tc.tile_pool(name="small", bufs=2))

        # --- persistent inputs / constants ---
        at_sb = cpool.tile([5, QPC], f32)
        nc.sync.dma_start(at_sb[:], at[:])
        bt_sb = cpool.tile([5, N], f32)
        nc.sync.dma_start(bt_sb[:], bt[:])

        if mm_bf16:
            bf16 = mybir.dt.bfloat16
            at_mm = cpool.tile([5, QPC], bf16)
            nc.vector.tensor_copy(at_mm[:], at_sb[:])
            bt_mm = cpool.tile([5, N], bf16)
            nc.vector.tensor_copy(bt_mm[:], bt_sb[:])
        else:
            at_mm, bt_mm = at_sb, bt_sb

        iota_u = cpool.tile([RT, CHUNK], u16)
        nc.gpsimd.iota(iota_u[:], pattern=[[-1, CHUNK]], base=CHUNK,
                       channel_multiplier=0)
        iota16 = cpool.tile([RT, CHUNK], f16)
        nc.gpsimd.tensor_copy(iota16[:], iota_u[:])

        acc = cpool.tile([RT, NRT], f32)
        flagacc = cpool.tile([RT, 1], f32)
        nc.vector.memset(flagacc[:], 0.0)

        rep_ctx = tc.For_i(0, repeat, 1) if repeat > 1 else None
        if rep_ctx is not None:
            rep_ctx.__enter__()

        for rt in range(NRT):
            P = P_BANDS[rt]
            nch = (P + CHUNK - 1) // CHUNK
            ntail = (N - P) // BLK

            # --- scores -> relu16; tail blocks also accumulate relu sums ---
            sgn = kpool.tile([RT, N], f16, tag="sgn")
            racc = spool.tile([RT, 16], f32, tag="racc")
            for g in range(N // BLK):
                ps = ppool.tile([RT, BLK], f32, tag="ps")
                nc.tensor.matmul(
                    out=ps[:],
                    lhsT=at_mm[:, rt * RT:(rt + 1) * RT],
                    rhs=bt_mm[:, g * BLK:(g + 1) * BLK],
                    start=True, stop=True,
                )
                # relu(score*1e30): in-ball -> +inf (f16), out -> exact 0.
                # fp32 score granularity (~1.5e-8) guarantees saturation.
                is_tail = g * BLK >= P
                nc.scalar.activation(
                    out=sgn[:, g * BLK:(g + 1) * BLK], in_=ps[:],
                    func=Act.Relu, scale=1e30,
                    accum_out=racc[:, g:g + 1] if is_tail else None,
                )

            # --- keys over the prefix: min(relu16, iota) (f16, 2x mode) ---
            keys = kpool.tile([RT, N], f16, tag="keys")
            for c in range(nch):
                w = min(CHUNK, P - c * CHUNK)
                nc.vector.tensor_tensor(
                    out=keys[:, c * CHUNK:c * CHUNK + w],
                    in0=sgn[:, c * CHUNK:c * CHUNK + w],
                    in1=iota16[:, :w], op=Alu.min,
                )

            # --- first-16 per chunk: max8, zap, max8 again ---
            cand = spool.tile([RT, 4 * 16], f16, tag="cand")
            for c in range(nch):
                w = min(CHUNK, P - c * CHUNK)
                kc = keys[:, c * CHUNK:c * CHUNK + w]
                s = c * 16
                nc.vector.max(out=cand[:, s:s + 8], in_=kc)
                nc.vector.match_replace(out=kc, in_to_replace=cand[:, s:s + 8],
                                        in_values=kc, imm_value=0.0)
                nc.vector.max(out=cand[:, s + 8:s + 16], in_=kc)

            # --- rebase chunk-local keys to global (descending in j) ---
            vplus = spool.tile([RT, 4 * 16], f32, tag="vplus")
            wk = spool.tile([RT, 4 * 16], f32, tag="wk")
            for c in range(nch):
                s = c * 16
                off = float(N - CHUNK * (c + 1))
                nc.vector.tensor_scalar(vplus[:, s:s + 16], cand[:, s:s + 16],
                                        off, None, Alu.add)
                nc.vector.scalar_tensor_tensor(
                    out=wk[:, s:s + 16], in0=cand[:, s:s + 16], scalar=0.0,
                    in1=vplus[:, s:s + 16], op0=Alu.is_gt, op1=Alu.mult,
                )

            # --- global top-16 of the candidates ---
            wv = wk[:, :nch * 16]
            wtop = spool.tile([RT, 16], f32, tag="wtop")
            nc.vector.max(out=wtop[:, 0:8], in_=wv)
            nc.vector.match_replace(out=wv, in_to_replace=wtop[:, 0:8],
                                    in_values=wv, imm_value=0.0)
            nc.vector.max(out=wtop[:, 8:16], in_=wv)

            # --- verification: flag queries with <16 found and nonempty tail ---
            if ntail > 0:
                tsum = spool.tile([RT, 1], f32, tag="tsum")
                nc.vector.tensor_reduce(out=tsum[:], in_=racc[:, 16 - ntail:],
                                        axis=mybir.AxisListType.X, op=Alu.add)
                incomplete = spool.tile([RT, 1], f32, tag="incomplete")
                nc.vector.tensor_scalar(incomplete[:], wtop[:, 15:16], 0.0,
                                        None, Alu.is_le)
                hastail = spool.tile([RT, 1], f32, tag="hastail")
                nc.vector.tensor_scalar(hastail[:], tsum[:], 0.0,
                                        None, Alu.is_gt)
                fl = spool.tile([RT, 1], f32, tag="fl")
                nc.vector.tensor_tensor(out=fl[:], in0=incomplete[:],
                                        in1=hastail[:], op=Alu.mult)
                nc.vector.tensor_tensor(out=flagacc[:], in0=flagacc[:],
                                        in1=fl[:], op=Alu.add)

            # --- indices: idx = N - w; pad invalid slots with first neighbor ---
            valid = spool.tile([RT, 16], i32, tag="valid")
            nc.vector.tensor_scalar(valid[:], wtop[:], 0.0, None, Alu.is_gt)
            idxf = spool.tile([RT, 16], f32, tag="idxf")
            nc.vector.tensor_scalar(idxf[:], wtop[:], -1.0, float(N),
                                    Alu.mult, Alu.add)
            idxp = spool.tile([RT, 16], f32, tag="idxp")
            nc.vector.tensor_copy(idxp[:], idxf[:, 0:1].to_broadcast([RT, 16]))
            nc.vector.copy_predicated(idxp[:], valid[:], idxf[:])
            nc.vector.tensor_scalar_min(idxp[:], idxp[:], float(N - 1))
            idx = spool.tile([RT, 16], i32, tag="idx")
            nc.vector.tensor_copy(idx[:], idxp[:])

            # --- gather neighbor flows (one offset per partition per DMA) ---
            nn = spool.tile([RT, K * 3], f32, tag="nn")
            if do_gather:
                for k in range(K):
                    nc.gpsimd.indirect_dma_start(
                        out=nn[:, k * 3:(k + 1) * 3], out_offset=None,
                        in_=flowall[:],
                        in_offset=bass.IndirectOffsetOnAxis(ap=idx[:, k:k + 1],
                                                            axis=0),
                    )
            else:
                nc.vector.tensor_copy(nn[:, 0:16], idxp[:])
                nc.vector.memset(nn[:, 16:], 0.5)
            fq = spool.tile([RT, 3], f32, tag="fq")
            nc.sync.dma_start(fq[:], flowq[rt * RT:(rt + 1) * RT, :])

            dif = spool.tile([RT, K * 3], f32, tag="dif")
            nn3 = nn[:].rearrange("p (k d) -> p k d", d=3)
            dif3 = dif[:].rearrange("p (k d) -> p k d", d=3)
            for dd in range(3):
                nc.vector.tensor_scalar(dif3[:, :, dd], nn3[:, :, dd],
                                        fq[:, dd:dd + 1], None, Alu.subtract)
            nc.vector.tensor_reduce(
                out=acc[:, rt:rt + 1], in_=dif[:], axis=mybir.AxisListType.X,
                op=Alu.add, apply_absolute_value=True,
            )

        if rep_ctx is not None:
            rep_ctx.__exit__(None, None, None)

        # --- final reductions ---
        accsum = cpool.tile([RT, 1], f32)
        nc.vector.tensor_reduce(out=accsum[:], in_=acc[:],
                                axis=mybir.AxisListType.X, op=Alu.add)
        tot = cpool.tile([RT, 1], f32)
        nc.gpsimd.partition_all_reduce(tot[:], accsum[:], channels=RT,
                                       reduce_op=bass_isa.ReduceOp.add)
        nc.sync.dma_start(partial[:], tot[0:1, :])

        fltot = cpool.tile([RT, 1], f32)
        nc.gpsimd.partition_all_reduce(fltot[:], flagacc[:], channels=RT,
                                       reduce_op=bass_isa.ReduceOp.add)
        nc.sync.dma_start(flags[:], fltot[0:1, :])

    nc.compile()
    return nc


def _get_program():
    global _cached
    if _cached is None:
        _cached = _build_program()
    return _cached


def _numpy_fallback(pc, flow):
    """Exact reference-semantics recompute on host (correctness backstop)."""
    total = 0.0
    r2 = 0.25
    for b in range(B):
        p = pc[b].astype(np.float32)
        sq = (p * p).sum(-1)
        for i in range(N):
            d2 = sq[i] + sq - 2.0 * (p @ p[i])
            ib = np.flatnonzero(d2 < r2)[:K]
            idx = np.concatenate([ib, np.full(K - len(ib), ib[0], np.int64)])
            total += np.abs(flow[b, i][None, :] - flow[b, idx]).sum(
                dtype=np.float64)
    return np.float32(total / (B * N * K))


def kernel(pc: np.ndarray, flow: np.ndarray) -> np.ndarray:
    from concourse.bass_utils import run_bass_kernel_spmd

    pc = np.asarray(pc, dtype=np.float32)
    flow = np.asarray(flow, dtype=np.float32)

    nc = _get_program()

    r2 = np.float32(RADIUS * RADIUS)
    sq = (pc * pc).sum(axis=-1, dtype=np.float32)  # [B, N]
    # density-sorted dealing: band t gets ranks [t*512, (t+1)*512), split
    # over the batch's 4 cores in 128-query row-tiles
    orders = [np.argsort(sq[b], kind="stable") for b in range(B)]

    in_maps = []
    for core in range(N_CORES):
        b = core // (N_CORES // B)
        csub = core % (N_CORES // B)
        perm = np.concatenate([
            orders[b][t * 512 + csub * RT: t * 512 + (csub + 1) * RT]
            for t in range(NRT)
        ])
        q = pc[b, perm]                     # [QPC, 3]
        at = np.concatenate(
            [q.T, sq[b, perm][None, :], np.ones((1, QPC), np.float32)], axis=0
        ).astype(np.float32)                # [5, QPC]
        p = pc[b]                           # [N, 3]
        bt = np.concatenate(
            [p.T, np.full((1, N), -0.5, np.float32),
             ((r2 - sq[b]) * np.float32(0.5))[None, :]], axis=0
        ).astype(np.float32)                # [5, N]
        in_maps.append({
            "at": np.ascontiguousarray(at),
            "bt": np.ascontiguousarray(bt),
            "flowall": np.ascontiguousarray(flow[b]),
            "flowq": np.ascontiguousarray(flow[b, perm]),
        })

    res = run_bass_kernel_spmd(nc, in_maps, list(range(N_CORES)))

    flagged = sum(float(res.results[c]["flags"].reshape(()))
                  for c in range(N_CORES))
    if flagged > 0:
        return _numpy_fallback(pc, flow)

    total = np.float32(0.0)
    for core in range(N_CORES):
        total += res.results[core]["partial"].reshape(())
    return np.float32(total / np.float32(B * N * K))


# revision 27
# speedup vs baseline: 1.7167x; 1.7167x over previous
"""BallQLoss kernel for 8 Trainium2 NeuronCores.

Computes mean_{b,i,k} |flow[b,i] - flow[b, idx[b,i,k]]|_1 where idx are the
first K=16 in-ball (radius 0.5) neighbors of each point in index order,
padded with the first neighbor (pointnet2 ball_query semantics).

Sharding: data-parallel over (B x N): each of 8 cores takes 2048 queries of
one batch element and holds the full 8192-point replica of that batch.

Queries are sorted by |q|^2 on the host (densest first) and dealt so that
row-tile t on every core holds queries of the same density band. Selection
then only scans a per-band prefix P_t of the index axis; a free tail-check
(ACT relu-sum accumulator) verifies on device that no query needed points
beyond its prefix, and the host falls back to an exact numpy computation in
that (never observed) case.

Per row-tile pipeline:
  PE    : score = (r^2 - d^2)/2 via an augmented 5-dim matmul (fp32)
  ACT   : relu(score * 1e30) -> f16 {inf, 0} + per-block accum for the tail
  DVE   : keys = min(relu16, iota_desc) (f16 2x); per-2048-chunk max8 ->
          match_replace -> max8 = first-16; rebase, merge, idx = N - key
  GPSIMD: per-slot indirect-DMA gather of neighbor flows
  DVE   : L1 diff reduce; partition-reduce partials at the end
"""

import numpy as np
from contextlib import ExitStack

K = 16
RADIUS = 0.5
B = 2
N = 8192
N_CORES = 8
QPC = (B * N) // N_CORES  # 2048 queries per core
RT = 128                  # queries per row-tile (SBUF partition dim)
NRT = QPC // RT           # 16 row-tiles per core
CHUNK = 2048              # fp16-exact local iota range
BLK = 512                 # PSUM bank width (fp32)

# Prefix length per density band (band = row-tile index after sorting by
# |q|^2 ascending within each batch). Measured max "needed prefix" on the
# reference input distribution + >=256 margin, rounded up to 512. The device
# verifies sufficiency at runtime; host falls back to numpy if flagged.
P_BANDS = [1536, 1536, 2048, 2048, 2048, 2560, 3072, 3584,
           3584, 4608, 5120, 7168, 8192, 8192, 8192, 8192]

_cached = None


def _build_program(repeat=1, mm_bf16=False, do_gather=True):
    import concourse.bass as bass
    import concourse.tile as tile
    from concourse import bacc, bass_isa, mybir

    f32 = mybir.dt.float32
    f16 = mybir.dt.float16
    i32 = mybir.dt.int32
    u16 = mybir.dt.uint16
    Alu = mybir.AluOpType
    Act = mybir.ActivationFunctionType

    nc = bacc.Bacc("TRN2", target_bir_lowering=False, debug=False,
                   num_devices=N_CORES)

    at = nc.dram_tensor("at", [5, QPC], f32, kind="ExternalInput").ap()
    bt = nc.dram_tensor("bt", [5, N], f32, kind="ExternalInput").ap()
    flowall = nc.dram_tensor("flowall", [N, 3], f32, kind="ExternalInput").ap()
    flowq = nc.dram_tensor("flowq", [QPC, 3], f32, kind="ExternalInput").ap()
    partial = nc.dram_tensor("partial", [1, 1], f32, kind="ExternalOutput").ap()
    flags = nc.dram_tensor("flags", [1, 1], f32, kind="ExternalOutput").ap()

    with tile.TileContext(nc) as tc, ExitStack() as ctx:
        cpool = ctx.enter_context(tc.tile_pool(name="const", bufs=1))
        kpool = ctx.enter_context(tc.tile_pool(name="keys", bufs=4))
        ppool = ctx.enter_context(tc.tile_pool(name="ps", bufs=8, space="PSUM"))
        spool = ctx.enter_context(tc.tile_pool(name="small", bufs=4))

        # --- persistent inputs / constants ---
        at_sb = cpool.tile([5, QPC], f32)
        nc.sync.dma_start(at_sb[:], at[:])
        bt_sb = cpool.tile([5, N], f32)
        nc.sync.dma_start(bt_sb[:], bt[:])

        if mm_bf16:
            bf16 = mybir.dt.bfloat16
            at_mm = cpool.tile([5, QPC], bf16)
            nc.vector.tensor_copy(at_mm[:], at_sb[:])
            bt_mm = cpool.tile([5, N], bf16)
            nc.vector.tensor_copy(bt_mm[:], bt_sb[:])
        else:
            at_mm, bt_mm = at_sb, bt_sb

        iota_u = cpool.tile([RT, CHUNK], u16)
        nc.gpsimd.iota(iota_u[:], pattern=[[-1, CHUNK]], base=CHUNK,
                       channel_multiplier=0)
        iota16 = cpool.tile([RT, CHUNK], f16)
        nc.gpsimd.tensor_copy(iota16[:], iota_u[:])

        acc = cpool.tile([RT, NRT], f32)
        flagacc = cpool.tile([RT, 1], f32)
        nc.vector.memset(flagacc[:], 0.0)

        rep_ctx = tc.For_i(0, repeat, 1) if repeat > 1 else None
        if rep_ctx is not None:
            rep_ctx.__enter__()

        for rt in range(NRT):
            P = P_BANDS[rt]
            nch = (P + CHUNK - 1) // CHUNK
            ntail = (N - P) // BLK

            # --- scores -> relu16; tail blocks also accumulate relu sums ---
            sgn = kpool.tile([RT, N], f16, tag="sgn")
            racc = spool.tile([RT, 16], f32, tag="racc")
            for g in range(N // BLK):
                ps = ppool.tile([RT, BLK], f32, tag="ps")
                nc.tensor.matmul(
                    out=ps[:],
                    lhsT=at_mm[:, rt * RT:(rt + 1) * RT],
                    rhs=bt_mm[:, g * BLK:(g + 1) * BLK],
                    start=True, stop=True,
                )
                # relu(score*1e30): in-ball -> +inf (f16), out -> exact 0.
                # fp32 score granularity (~1.5e-8) guarantees saturation.
                is_tail = g * BLK >= P
                nc.scalar.activation(
                    out=sgn[:, g * BLK:(g + 1) * BLK], in_=ps[:],
                    func=Act.Relu, scale=1e30,
                    accum_out=racc[:, g:g + 1] if is_tail else None,
                )

            # --- keys over the prefix: min(relu16, iota) (f16, 2x mode) ---
            keys = kpool.tile([RT, N], f16, tag="keys")
            for c in range(nch):
                w = min(CHUNK, P - c * CHUNK)
                nc.vector.tensor_tensor(
                    out=keys[:, c * CHUNK:c * CHUNK + w],
                    in0=sgn[:, c * CHUNK:c * CHUNK + w],
                    in1=iota16[:, :w], op=Alu.min,
                )

            # --- first-16 per chunk: max8, zap, max8 again ---
            cand = spool.tile([RT, 4 * 16], f16, tag="cand")
            for c in range(nch):
                w = min(CHUNK, P - c * CHUNK)
                kc = keys[:, c * CHUNK:c * CHUNK + w]
                s = c * 16
                nc.vector.max(out=cand[:, s:s + 8], in_=kc)
                nc.vector.match_replace(out=kc, in_to_replace=cand[:, s:s + 8],
                                        in_values=kc, imm_value=0.0)
                nc.vector.max(out=cand[:, s + 8:s + 16], in_=kc)

            # --- rebase chunk-local keys to global (descending in j) ---
            vplus = spool.tile([RT, 4 * 16], f32, tag="vplus")
            wk = spool.tile([RT, 4 * 16], f32, tag="wk")
            for c in range(nch):
                s = c * 16
                off = float(N - CHUNK * (c + 1))
                nc.vector.tensor_scalar(vplus[:, s:s + 16], cand[:, s:s + 16],
                                        off, None, Alu.add)
                nc.vector.scalar_tensor_tensor(
                    out=wk[:, s:s + 16], in0=cand[:, s:s + 16], scalar=0.0,
                    in1=vplus[:, s:s + 16], op0=Alu.is_gt, op1=Alu.mult,
                )

            # --- global top-16 of the candidates ---
            wv = wk[:, :nch * 16]
            wtop = spool.tile([RT, 16], f32, tag="wtop")
            nc.vector.max(out=wtop[:, 0:8], in_=wv)
            nc.vector.match_replace(out=wv, in_to_replace=wtop[:, 0:8],
                                    in_values=wv, imm_value=0.0)
            nc.vector.max(out=wtop[:, 8:16], in_=wv)

            # --- verification: flag queries with <16 found and nonempty tail ---
            if ntail > 0:
                tsum = spool.tile([RT, 1], f32, tag="tsum")
                nc.vector.tensor_reduce(out=tsum[:], in_=racc[:, 16 - ntail:],
                                        axis=mybir.AxisListType.X, op=Alu.add)
                incomplete = spool.tile([RT, 1], f32, tag="incomplete")
                nc.vector.tensor_scalar(incomplete[:], wtop[:, 15:16], 0.0,
                                        None, Alu.is_le)
                hastail = spool.tile([RT, 1], f32, tag="hastail")
                nc.vector.tensor_scalar(hastail[:], tsum[:], 0.0,
                                        None, Alu.is_gt)
                fl = spool.tile([RT, 1], f32, tag="fl")
                nc.vector.tensor_tensor(out=fl[:], in0=incomplete[:],
                                        in1=hastail[:], op=Alu.mult)
                nc.vector.tensor_tensor(out=flagacc[:], in0=flagacc[:],
                                        in1=fl[:], op=Alu.add)

            # --- indices: idx = N - w; pad invalid slots with first neighbor ---
            valid = spool.tile([RT, 16], i32, tag="valid")
            nc.vector.tensor_scalar(valid[:], wtop[:], 0.0, None, Alu.is_gt)
            idxf = spool.tile([RT, 16], f32, tag="idxf")
            nc.vector.tensor_scalar(idxf[:], wtop[:], -1.0, float(N),
                                    Alu.mult, Alu.add)
            idxp = spool.tile([RT, 16], f32, tag="idxp")
            nc.vector.tensor_copy(idxp[:], idxf[:, 0:1].to_broadcast([RT, 16]))
            nc.vector.copy_predicated(idxp[:], valid[:], idxf[:])
            nc.vector.tensor_scalar_min(idxp[:], idxp[:], float(N - 1))
            idx = spool.tile([RT, 16], i32, tag="idx")
            nc.vector.tensor_copy(idx[:], idxp[:])

            # --- gather neighbor flows (one offset per partition per DMA) ---
            nn = spool.tile([RT, K * 3], f32, tag="nn")
            if do_gather:
                for k in range(K):
                    nc.gpsimd.indirect_dma_start(
                        out=nn[:, k * 3:(k + 1) * 3], out_offset=None,
                        in_=flowall[:],
                        in_offset=bass.IndirectOffsetOnAxis(ap=idx[:, k:k + 1],
                                                            axis=0),
                    )
            else:
                nc.vector.tensor_copy(nn[:, 0:16], idxp[:])
                nc.vector.memset(nn[:, 16:], 0.5)
            fq = spool.tile([RT, 3], f32, tag="fq")
            nc.sync.dma_start(fq[:], flowq[rt * RT:(rt + 1) * RT, :])

            dif = spool.tile([RT, K * 3], f32, tag="dif")
            nn3 = nn[:].rearrange("p (k d) -> p k d", d=3)
            dif3 = dif[:].rearrange("p (k d) -> p k d", d=3)
            for dd in range(3):
                nc.vector.tensor_scalar(dif3[:, :, dd], nn3[:, :, dd],
                                        fq[:, dd:dd + 1], None, Alu.subtract)
            nc.vector.tensor_reduce(
                out=acc[:, rt:rt + 1], in_=dif[:], axis=mybir.AxisListType.X,
                op=Alu.add, apply_absolute_value=True,
            )

        if rep_ctx is not None:
            rep_ctx.__exit__(None, None, None)

        # --- final reductions ---
        accsum = cpool.tile([RT, 1], f32)
        nc.vector.tensor_reduce(out=accsum[:], in_=acc[:],
                                axis=mybir.AxisListType.X, op=Alu.add)
        tot = cpool.tile([RT, 1], f32)
        nc.gpsimd.partition_all_reduce(tot[:], accsum[:], channels=RT,
                                       reduce_op=bass_isa.ReduceOp.add)
        nc.sync.dma_start(partial[:], tot[0:1, :])

        fltot = cpool.tile([RT, 1], f32)
        nc.gpsimd.partition_all_reduce(fltot[:], flagacc[:], channels=RT,
                                       reduce_op=bass_isa.ReduceOp.add)
        nc.sync.dma_start(flags[:], fltot[0:1, :])

    nc.compile()
    return nc


def _get_program():
    global _cached
    if _cached is None:
        _cached = _build_program()
    return _cached


def _numpy_fallback(pc, flow):
    """Exact reference-semantics recompute on host (correctness backstop)."""
    total = 0.0
    r2 = 0.25
    for b in range(B):
        p = pc[b].astype(np.float32)
        sq = (p * p).sum(-1)
        for i in range(N):
            d2 = sq[i] + sq - 2.0 * (p @ p[i])
            ib = np.flatnonzero(d2 < r2)[:K]
            idx = np.concatenate([ib, np.full(K - len(ib), ib[0], np.int64)])
            total += np.abs(flow[b, i][None, :] - flow[b, idx]).sum(
                dtype=np.float64)
    return np.float32(total / (B * N * K))


def kernel(pc: np.ndarray, flow: np.ndarray) -> np.ndarray:
    from concourse.bass_utils import run_bass_kernel_spmd

    pc = np.asarray(pc, dtype=np.float32)
    flow = np.asarray(flow, dtype=np.float32)

    nc = _get_program()

    r2 = np.float32(RADIUS * RADIUS)
    sq = (pc * pc).sum(axis=-1, dtype=np.float32)  # [B, N]
    # density-sorted dealing: band t gets ranks [t*512, (t+1)*512), split
    # over the batch's 4 cores in 128-query row-tiles
    orders = [np.argsort(sq[b], kind="stable") for b in range(B)]

    in_maps = []
    for core in range(N_CORES):
        b = core // (N_CORES // B)
        csub = core % (N_CORES // B)
        perm = np.concatenate([
            orders[b][t * 512 + csub * RT: t * 512 + (csub + 1) * RT]
            for t in range(NRT)
        ])
        q = pc[b, perm]                     # [QPC, 3]
        at = np.concatenate(
            [q.T, sq[b, perm][None, :], np.ones((1, QPC), np.float32)], axis=0
        ).astype(np.float32)                # [5, QPC]
        p = pc[b]                           # [N, 3]
        bt = np.concatenate(
            [p.T, np.full((1, N), -0.5, np.float32),
             ((r2 - sq[b]) * np.float32(0.5))[None, :]], axis=0
        ).astype(np.float32)                # [5, N]
        in_maps.append({
            "at": np.ascontiguousarray(at),
            "bt": np.ascontiguousarray(bt),
            "flowall": np.ascontiguousarray(flow[b]),
            "flowq": np.ascontiguousarray(flow[b, perm]),
        })

    res = run_bass_kernel_spmd(nc, in_maps, list(range(N_CORES)))

    flagged = sum(float(res.results[c]["flags"].reshape(()))
                  for c in range(N_CORES))
    if flagged > 0:
        return _numpy_fallback(pc, flow)

    total = np.float32(0.0)
    for core in range(N_CORES):
        total += res.results[core]["partial"].reshape(())
    return np.float32(total / np.float32(B * N * K))
